# revision 1
# baseline (speedup 1.0000x reference)
"""Blockwise-parallel transformer attention on 8 TRN2 NeuronCores.

Reference computation (per batch b):
    k = x@Wk + bk ; v = x@Wv + bv            (from ORIGINAL x, layer-invariant)
    h = x
    6x (shared weights):
        q = h@Wq + bq
        P = softmax(q k^T / 8)
        attn = (P @ v) / sqrt(512)
        ff = relu(attn@W1 + b1)@W2 + b2
        h = LN2(LN1(h + ff))

Sharding: 8 cores = 4 batches x 2 query-halves. Each core computes full
k/v for its batch (once), then processes its 1024-query slice through all
6 layers with zero cross-core traffic.

On-chip layout is fully transposed (feature dim on partitions, tokens on
the free axis); the host feeds x^T so the device never transposes.
Softmax/LN reductions over the partition axis use ones-vector matmuls;
broadcasts back across partitions use the GPSIMD partition_broadcast
ucode instruction. The residual+bias add rides inside the ff matmul
group (identity and rank-1 matmuls), so PSUM already holds h+ff+b2.
bq folds into the exp bias via ck = (k@bq)/8 (k is layer-invariant).

When the LayerNorm affine params are trivial (g=1, b=0 — checked at
runtime), LN2(LN1(t)) collapses to (t-mu1)*(r1*r2) with r2 a scalar
function of v1, halving the normalization work (specialized program).
"""

import sys

if "/opt/trn_rl_repo" not in sys.path:
    sys.path.insert(0, "/opt/trn_rl_repo")

import numpy as np
import ml_dtypes

import concourse.bass as bass
import concourse.mybir as mybir
import concourse.tile as tile
from concourse import bacc
import concourse.hw_specs as _hw_specs


def _restrict_act_tables():
    """All activation functions this kernel uses (exp, ln, relu, copy)
    live in the natural_log_exp_and_others table set. Left to its own
    devices the table-load pass alternates between exp_and_others and the
    ln set (~49 reloads x 1.5us of ACT time per run); restricting the
    offered sets collapses that to a single load. Dict order is preserved
    so act_func_set_id stays aligned with act_info.json."""
    if getattr(_hw_specs, "_act_tables_restricted", False):
        return
    orig = _hw_specs.get_activation_tables

    def restricted(arch):
        tables = orig(arch)
        return {
            name: (fns if name == "natural_log_exp_and_others" else set())
            for name, fns in tables.items()
        }

    _hw_specs.get_activation_tables = restricted
    bacc.get_activation_tables = restricted
    _hw_specs._act_tables_restricted = True


_restrict_act_tables()
from concourse.bass_utils import run_bass_kernel_spmd
from concourse.masks import make_identity

F32 = mybir.dt.float32
BF16 = mybir.dt.bfloat16
F8 = mybir.dt.float8e4
DR = mybir.MatmulPerfMode.DoubleRow
EXP = mybir.ActivationFunctionType.Exp
LN_ = mybir.ActivationFunctionType.Ln
RELU = mybir.ActivationFunctionType.Relu
ADD = mybir.AluOpType.add
SUB = mybir.AluOpType.subtract
MULT = mybir.AluOpType.mult

B, S, D, HID, L = 4, 2048, 512, 64, 6
EPS = 1e-5
P = 128


def build(S=S, SQ=S // 2, D=D, HID=HID, L=L, trivial_ln=False, trivial_bias=False):
    """Build + compile the per-core Bass program (same program on all 8 cores)."""
    C = D // P          # feature-dim 128-chunks (4)
    MK = S // P         # key-token 128-chunks (16)
    FK = min(512, S)    # key free-dim tile
    NK = S // FK
    FQ = min(512, SQ)   # query free-dim tile
    NQ = SQ // FQ
    scale_attn = 1.0 / float(np.sqrt(HID))
    scale_out = 1.0 / float(np.sqrt(D))

    nc = bacc.Bacc("TRN2", target_bir_lowering=False, debug=False)

    # ---- DRAM I/O (per core) ----
    xt = nc.dram_tensor("xt", (C, P, S), F8, kind="ExternalInput")
    xq = nc.dram_tensor("xq", (C, P, SQ), BF16, kind="ExternalInput")
    wq = nc.dram_tensor("wq", (C, P, D), F8, kind="ExternalInput")
    wk = nc.dram_tensor("wk", (C, P, D), F8, kind="ExternalInput")
    wv = nc.dram_tensor("wv", (C, P, D), F8, kind="ExternalInput")
    w1 = nc.dram_tensor("w1", (C, P, HID), BF16, kind="ExternalInput")
    w2 = nc.dram_tensor("w2", (HID, D), BF16, kind="ExternalInput")
    bqc = nc.dram_tensor("bqc", (C, P, 1), F8, kind="ExternalInput")
    bk = nc.dram_tensor("bk", (P, C), F32, kind="ExternalInput")
    bv = nc.dram_tensor("bv", (1, D), F32, kind="ExternalInput")
    b1d = nc.dram_tensor("b1d", (HID, 1), F32, kind="ExternalInput")
    b2r = nc.dram_tensor("b2r", (1, D), BF16, kind="ExternalInput")
    g1d = nc.dram_tensor("g1d", (P, C), F32, kind="ExternalInput")
    be1d = nc.dram_tensor("be1d", (P, C), F32, kind="ExternalInput")
    g2d = nc.dram_tensor("g2d", (P, C), F32, kind="ExternalInput")
    be2d = nc.dram_tensor("be2d", (P, C), F32, kind="ExternalInput")
    out = nc.dram_tensor("out", (C, P, SQ), F32, kind="ExternalOutput")

    with tile.TileContext(nc) as tc:
        with (
            tc.tile_pool(name="const", bufs=1) as cons,
            tc.tile_pool(name="big", bufs=1) as big,
            tc.tile_pool(name="vec", bufs=2) as vecp,
            tc.tile_pool(name="psA", bufs=6, space="PSUM") as psA,
            tc.tile_pool(name="psS", bufs=2, space="PSUM") as psS,
        ):
            # ---- persistent SBUF ----
            wq_sb = cons.tile([P, C, D], F8)
            wk_sb = cons.tile([P, C, D], F8)
            wv_sb = cons.tile([P, C, D], F8)
            w1_sb = cons.tile([P, C, HID], BF16)
            w2_sb = cons.tile([HID, D], BF16)
            bqc_sb = cons.tile([P, C], F8)
            bk_sb = cons.tile([P, C], F32)
            bv_sb = cons.tile([1, D], F32)
            bv_bc = cons.tile([P, D], F32)
            b1_sb = cons.tile([HID, 1], F32)
            b2r_sb = cons.tile([1, D], BF16)
            g1_sb = cons.tile([P, C], F32)
            be1_sb = cons.tile([P, C], F32)
            g2_sb = cons.tile([P, C], F32)
            be2_sb = cons.tile([P, C], F32)
            ones_bf = cons.tile([P, 1], BF16)
            ones2_f8 = cons.tile([P, 2, 16], F8)  # pair-dim stride must be 16B
            ones_row = cons.tile([1, SQ], BF16)
            eps_sb = cons.tile([1, 1], F32)
            eps2_sb = cons.tile([1, 1], F32)
            ident_sb = cons.tile([P, P], BF16)
            ck_sb = cons.tile([P, MK], F32)   # exp bias: (k @ bq)/8 per key token

            k_sb = cons.tile([P, C, S], F8)       # k^T (fp8 for DoubleRow)
            v_sb = cons.tile([P, MK, D], F8)      # v natural (fp8)
            vbf_sb = None
            if not trivial_bias:
                vbf_sb = cons.tile([P, MK, D], BF16)
            h_sb = cons.tile([P, C, SQ], BF16)    # h^T (residual stream)
            hf8_sb = cons.tile([P, C, SQ], F8)    # h^T in fp8 for the q matmul
            q_sb = cons.tile([P, C, SQ], F8)      # q^T (fp8)
            attn_sb = cons.tile([P, C, SQ], BF16)
            r_sb = cons.tile([HID, SQ], BF16)     # relu(ffn hidden)
            t_sb = cons.tile([P, C, SQ], BF16)    # residual pre-LN / h1
            tsq_sb = cons.tile([P, C, SQ], BF16)
            stw_sb = cons.tile([P, 2, SQ], BF16)  # [sum(t), sum(t^2)] over C
            hout_sb = cons.tile([P, C, SQ], F32)  # final-layer f32 output
            recip_bc = cons.tile([P, SQ], BF16)
            mu1_bc = cons.tile([P, SQ], BF16)
            rstd1_bc = cons.tile([P, SQ], BF16)
            mu2_bc = cons.tile([P, SQ], BF16)
            rstd2_bc = cons.tile([P, SQ], BF16)

            # xt is setup-only; share its slot with the (larger) P matrix
            xt_sb = big.tile([P, C, S], F8, tag="bigshare")
            # ---- load constants & inputs ----
            for c in range(C):
                nc.sync.dma_start(wk_sb[:, c, :], wk[c, :, :])
                nc.sync.dma_start(wv_sb[:, c, :], wv[c, :, :])
            for c in range(C):
                for pc in range(2):
                    sl = slice(pc * (S // 2), (pc + 1) * (S // 2))
                    nc.sync.dma_start(xt_sb[:, c, sl], xt[c, :, sl])
            for c in range(C):
                nc.sync.dma_start(wq_sb[:, c, :], wq[c, :, :])
            nc.sync.dma_start(w1_sb[:], w1[:, :, :].rearrange("c p d -> p c d"))
            nc.sync.dma_start(w2_sb[:], w2[:, :])
            nc.sync.dma_start(bqc_sb[:], bqc[:, :, 0].rearrange("c p -> p c"))
            nc.sync.dma_start(bk_sb[:], bk[:, :])
            nc.sync.dma_start(bv_sb[:], bv[:, :])
            nc.sync.dma_start(b1_sb[:], b1d[:, :])
            nc.sync.dma_start(b2r_sb[:], b2r[:, :])
            nc.sync.dma_start(g1_sb[:], g1d[:, :])
            nc.sync.dma_start(be1_sb[:], be1d[:, :])
            nc.sync.dma_start(g2_sb[:], g2d[:, :])
            nc.sync.dma_start(be2_sb[:], be2d[:, :])
            for c in range(C):
                nc.sync.dma_start(h_sb[:, c, :], xq[c, :, :])
            nc.vector.memset(ones_bf[:], 1.0)
            nc.vector.memset(ones2_f8[:], 1.0)
            nc.vector.memset(ones_row[:], 1.0)
            nc.vector.memset(eps_sb[:], EPS)
            nc.vector.memset(eps2_sb[:], EPS * EPS)
            make_identity(nc, ident_sb[:])
            nc.vector.tensor_copy(hf8_sb[:], h_sb[:])
            nc.gpsimd.partition_broadcast(bv_bc[:], bv_sb[0:1, :])
            # HAM warmup: keep the PE busy while the input DMAs land so the
            # first real matmuls run at full clock
            wu = psA.tile([P, P], F32, tag="main")
            for _ in range(40):
                nc.tensor.matmul(wu[:], ident_sb[:], ident_sb[:],
                                 start=True, stop=True)

            # ---- k^T = Wk^T x^T + bk ----
            for nk in range(NK):
                for c in range(C):
                    ps = psA.tile([P, FK], F32, tag="main")
                    for t2 in range(C // 2):
                        nc.tensor.matmul(
                            ps[:],
                            wk_sb[:, 2 * t2:2 * t2 + 2, c * P:(c + 1) * P],
                            xt_sb[:, 2 * t2:2 * t2 + 2, nk * FK:(nk + 1) * FK],
                            start=(t2 == 0),
                            stop=(t2 == C // 2 - 1),
                            perf_mode=DR,
                        )
                    nc.scalar.activation(
                        k_sb[:, c, nk * FK:(nk + 1) * FK], ps[:],
                        mybir.ActivationFunctionType.Identity,
                        bias=bk_sb[:, c:c + 1],
                    )
                # fill the DMA wait for the next token-range pieces and keep
                # the PE warm
                for _ in range(6):
                    nc.tensor.matmul(wu[:], ident_sb[:], ident_sb[:],
                                     start=True, stop=True)

            # ---- ck = (k @ bq) * scale_attn  (exp bias; layer-invariant) ----
            for mk in range(MK) if not trivial_bias else []:
                ps = psS.tile([P, 1], F32, tag="stat")
                for c in range(C):
                    nc.tensor.matmul(
                        ps[:],
                        k_sb[:, c, mk * P:(mk + 1) * P],
                        bqc_sb[:, c:c + 1],
                        start=(c == 0),
                        stop=(c == C - 1),
                    )
                nc.vector.tensor_scalar_mul(ck_sb[:, mk:mk + 1], ps[:], scale_attn)

            # fp8 P is safe only with zero biases (logits stay in ~[-3,3]);
            # the general path keeps bf16 P and standard matmuls
            P_dt = F8 if trivial_bias else BF16
            P_sb = cons.tile([P, MK, SQ], P_dt)  # exp(scores^T)

            def emit_v_setup():
                # emitted after the first scores block: v is only needed by
                # attention, so this keeps the wv/xt DMAs off the critical
                # startup path
                for mk in range(MK):
                    ps = psA.tile([P, D], F32, tag="main")
                    for t2 in range(C // 2):
                        nc.tensor.matmul(
                            ps[:],
                            xt_sb[:, 2 * t2:2 * t2 + 2, mk * P:(mk + 1) * P],
                            wv_sb[:, 2 * t2:2 * t2 + 2, :],
                            start=(t2 == 0),
                            stop=(t2 == C // 2 - 1),
                            perf_mode=DR,
                        )
                    nc.vector.tensor_tensor(v_sb[:, mk, :], ps[:], bv_bc[:], ADD)
                    if not trivial_bias:
                        nc.vector.tensor_tensor(vbf_sb[:, mk, :], ps[:],
                                                bv_bc[:], ADD)

            def layer_norm(src, dst, g, be, mu_bc, rstd_bc, nq, out_f32=False,
                           use_stw=False):
                """General LN over the feature axis for token chunk nq.
                use_stw: the ff loop prebuilt sum(t)/sum(t^2) into stw_sb
                (valid for LN1 only; LN2 recomputes from its input)."""
                ts = slice(nq * FQ, (nq + 1) * FQ)
                if not use_stw:
                    nc.vector.tensor_mul(tsq_sb[:, :, ts], src[:, :, ts],
                                         src[:, :, ts])
                    nc.vector.tensor_tensor(
                        stw_sb[:, 0:1, ts], src[:, 0:1, ts], src[:, 1:2, ts], ADD)
                    nc.vector.tensor_tensor(
                        stw_sb[:, 0:1, ts], stw_sb[:, 0:1, ts], src[:, 2:3, ts], ADD)
                    nc.vector.tensor_tensor(
                        stw_sb[:, 0:1, ts], stw_sb[:, 0:1, ts], src[:, 3:4, ts], ADD)
                    nc.vector.tensor_tensor(
                        stw_sb[:, 1:2, ts], tsq_sb[:, 0:1, ts], tsq_sb[:, 1:2, ts], ADD)
                    nc.vector.tensor_tensor(
                        stw_sb[:, 1:2, ts], stw_sb[:, 1:2, ts], tsq_sb[:, 2:3, ts], ADD)
                    nc.vector.tensor_tensor(
                        stw_sb[:, 1:2, ts], stw_sb[:, 1:2, ts], tsq_sb[:, 3:4, ts], ADD)
                ps1 = psS.tile([1, FQ], F32, tag="stat")
                nc.tensor.matmul(ps1[:], ones_bf[:], stw_sb[:, 0, ts],
                                 start=True, stop=True)
                ps2 = psS.tile([1, FQ], F32, tag="stat")
                nc.tensor.matmul(ps2[:], ones_bf[:], stw_sb[:, 1, ts],
                                 start=True, stop=True)
                mu = vecp.tile([1, FQ], BF16, tag="v1")
                ev = vecp.tile([1, FQ], F32, tag="v2")
                msq = vecp.tile([1, FQ], F32, tag="v3")
                rstd = vecp.tile([1, FQ], BF16, tag="v4")
                nc.vector.tensor_scalar_mul(mu[:], ps1[:], 1.0 / D)
                nc.vector.tensor_scalar_mul(ev[:], ps2[:], 1.0 / D)
                nc.vector.tensor_mul(msq[:], mu[:], mu[:])
                nc.vector.tensor_tensor(ev[:], ev[:], msq[:], SUB)
                nc.scalar.activation(ev[:], ev[:], LN_, bias=eps_sb[:])
                nc.scalar.activation(rstd[:], ev[:], EXP, scale=-0.5)
                nc.gpsimd.partition_broadcast(mu_bc[:, ts], mu[0:1, :])
                nc.gpsimd.partition_broadcast(rstd_bc[:, ts], rstd[0:1, :])
                bshape = (P, C, FQ)
                nc.vector.tensor_tensor(
                    dst[:, :, ts], src[:, :, ts],
                    mu_bc[:, None, ts].to_broadcast(bshape), SUB,
                )
                nc.vector.tensor_tensor(
                    dst[:, :, ts], dst[:, :, ts],
                    rstd_bc[:, None, ts].to_broadcast(bshape), MULT,
                )
                dd = hout_sb if out_f32 else dst
                for c in range(C):
                    nc.vector.tensor_scalar(
                        dd[:, c, ts], dst[:, c, ts],
                        g[:, c:c + 1], be[:, c:c + 1], MULT, ADD,
                    )
                    if out_f32:
                        nc.sync.dma_start(out[c, :, ts], hout_sb[:, c, ts])
                if not out_f32 and dst is not t_sb:
                    nc.vector.tensor_copy(hf8_sb[:, :, ts], dst[:, :, ts])

            def fused_trivial_ln(src, dst, nq, out_f32=False):
                """LN2(LN1(t)) with g=1,b=0: h = (t-mu1)*(r1*r2),
                r2 = rsqrt(v1/(v1+eps) + eps)."""
                ts = slice(nq * FQ, (nq + 1) * FQ)
                ps1 = psS.tile([1, FQ], F32, tag="stat")
                nc.tensor.matmul(ps1[:], ones_bf[:], stw_sb[:, 0, ts],
                                 start=True, stop=True)
                ps2 = psS.tile([1, FQ], F32, tag="stat")
                nc.tensor.matmul(ps2[:], ones_bf[:], stw_sb[:, 1, ts],
                                 start=True, stop=True)
                mu = vecp.tile([1, FQ], BF16, tag="v1")
                ev = vecp.tile([1, FQ], F32, tag="v2")
                msq = vecp.tile([1, FQ], F32, tag="v3")
                alpha = vecp.tile([1, FQ], BF16, tag="v6")
                nc.vector.tensor_scalar_mul(mu[:], ps1[:], 1.0 / D)
                nc.vector.tensor_scalar_mul(ev[:], ps2[:], 1.0 / D)
                nc.vector.tensor_mul(msq[:], mu[:], mu[:])
                nc.vector.tensor_tensor(ev[:], ev[:], msq[:], SUB)  # v1
                # r1*r2 = rsqrt((v1+eps)*(v2+eps)) with v2=v1/(v1+eps)
                #       = rsqrt(v1*(1+eps) + eps^2)  (exact algebra)
                nc.scalar.activation(ev[:], ev[:], LN_,
                                     bias=eps2_sb[:], scale=1.0 + EPS)
                nc.scalar.activation(alpha[:], ev[:], EXP, scale=-0.5)
                nc.gpsimd.partition_broadcast(mu1_bc[:, ts], mu[0:1, :])
                nc.gpsimd.partition_broadcast(rstd1_bc[:, ts], alpha[0:1, :])
                bshape = (P, C, FQ)
                dd = hout_sb if out_f32 else dst
                nc.vector.tensor_tensor(
                    dst[:, :, ts], src[:, :, ts],
                    mu1_bc[:, None, ts].to_broadcast(bshape), SUB,
                )
                nc.vector.tensor_tensor(
                    dd[:, :, ts], dst[:, :, ts],
                    rstd1_bc[:, None, ts].to_broadcast(bshape), MULT,
                )
                if out_f32:
                    for c in range(C):
                        nc.sync.dma_start(out[c, :, ts], hout_sb[:, c, ts])
                else:
                    nc.vector.tensor_copy(hf8_sb[:, :, ts], dst[:, :, ts])

            # ---- transformer layers ----
            pending_ln = []

            def emit_q(nq):
                # q^T = Wq^T h^T  (bq folded into the exp bias via ck)
                ts = slice(nq * FQ, (nq + 1) * FQ)
                for c in range(C):
                    ps = psA.tile([P, FQ], F32, tag="main")
                    for t2 in range(C // 2):
                        nc.tensor.matmul(
                            ps[:],
                            wq_sb[:, 2 * t2:2 * t2 + 2, c * P:(c + 1) * P],
                            hf8_sb[:, 2 * t2:2 * t2 + 2, ts],
                            start=(t2 == 0),
                            stop=(t2 == C // 2 - 1),
                            perf_mode=DR,
                        )
                    if c % 2 == 0:
                        nc.scalar.copy(q_sb[:, c, ts], ps[:])
                    else:
                        nc.vector.tensor_copy(q_sb[:, c, ts], ps[:])

            q_done = set()
            for li in range(L):
                last = li == L - 1
                # fully per-token-chunk pipeline: chunk nq's softmax/LN
                # chains hide under chunk nq+1's (and next layer's) matmuls
                half = MK // 2
                for nq in range(NQ):
                    ts = slice(nq * FQ, (nq + 1) * FQ)
                    if (li, nq) not in q_done:
                        emit_q(nq)
                        q_done.add((li, nq))
                    # scores^T = k q^T ; P = exp(scores*scale + ck)
                    for mk in range(MK):
                        ps = psA.tile([P, FQ], F32, tag="main")
                        for t2 in range(C // 2):
                            nc.tensor.matmul(
                                ps[:],
                                k_sb[:, 2 * t2:2 * t2 + 2, mk * P:(mk + 1) * P],
                                q_sb[:, 2 * t2:2 * t2 + 2, ts],
                                start=(t2 == 0),
                                stop=(t2 == C // 2 - 1),
                                perf_mode=DR,
                            )
                        nc.scalar.activation(
                            P_sb[:, mk, ts], ps[:], EXP,
                            bias=0.0 if trivial_bias else ck_sb[:, mk:mk + 1],
                            scale=scale_attn,
                        )
                    # previous chunk's LN flushes here: its stat matmuls sit
                    # behind this chunk's scores in the PE queue, so the
                    # stats' DVE feed chain is fully covered
                    while pending_ln:
                        pending_ln.pop(0)()
                    if li == 0 and nq == 0:
                        emit_v_setup()
                    # attn^T = v^T P^T, normalized by recip; the denominator
                    # colsum (fp8 DoubleRow ones-matmul) is slotted after the
                    # first attn group
                    for c in range(C):
                        ps = psA.tile([P, FQ], F32, tag="main")
                        if trivial_bias:
                            for t2 in range(MK // 2):
                                nc.tensor.matmul(
                                    ps[:],
                                    v_sb[:, 2 * t2:2 * t2 + 2, c * P:(c + 1) * P],
                                    P_sb[:, 2 * t2:2 * t2 + 2, ts],
                                    start=(t2 == 0),
                                    stop=(t2 == MK // 2 - 1),
                                    perf_mode=DR,
                                )
                        else:
                            for mk in range(MK):
                                nc.tensor.matmul(
                                    ps[:],
                                    vbf_sb[:, mk, c * P:(c + 1) * P],
                                    P_sb[:, mk, ts],
                                    start=(mk == 0),
                                    stop=(mk == MK - 1),
                                )
                        if c == 0:
                            psd = psS.tile([1, FQ], F32, tag="stat")
                            if trivial_bias:
                                for t2 in range(MK // 2):
                                    nc.tensor.matmul(
                                        psd[:], ones2_f8[:, :, 0:1],
                                        P_sb[:, 2 * t2:2 * t2 + 2, ts],
                                        start=(t2 == 0),
                                        stop=(t2 == MK // 2 - 1),
                                        perf_mode=DR,
                                    )
                            else:
                                for mk in range(MK):
                                    nc.tensor.matmul(
                                        psd[:], ones_bf[:], P_sb[:, mk, ts],
                                        start=(mk == 0),
                                        stop=(mk == MK - 1),
                                    )
                            den = vecp.tile([1, FQ], BF16, tag="vden")
                            dnl = vecp.tile([1, FQ], F32, tag="vdnl")
                            nc.scalar.activation(dnl[:], psd[:], LN_, bias=0.0)
                            nc.scalar.activation(den[:], dnl[:], EXP,
                                                 scale=-1.0)
                            nc.gpsimd.partition_broadcast(
                                recip_bc[:, ts], den[0:1, :])
                        if c % 2 == 0:
                            nc.scalar.copy(attn_sb[:, c, ts], ps[:])
                        else:
                            nc.vector.tensor_copy(attn_sb[:, c, ts], ps[:])
                    # ffn hidden: r = relu(attn@W1 * scale_out + b1)
                    ps = psA.tile([HID, FQ], F32, tag="main")
                    for kt in range(C):
                        nc.tensor.matmul(
                            ps[:], w1_sb[:, kt, :], attn_sb[:, kt, ts],
                            start=(kt == 0), stop=(kt == C - 1),
                        )
                    # prefetch the next chunk's q matmuls: their inputs are
                    # ready, and they fill the recip->relu chain drain
                    nxt = (li, nq + 1) if nq + 1 < NQ else (li + 1, 0)
                    if nxt[0] < L and nxt not in q_done:
                        emit_q(nxt[1])
                        q_done.add(nxt)
                    # softmax normalization applied here: the per-token recip
                    # commutes through the linear W1 matmul, so scaling the
                    # [64 x FQ] hidden is 8x cheaper than scaling attn
                    nc.vector.tensor_mul(ps[:], ps[:], recip_bc[:HID, ts])
                    nc.scalar.activation(
                        r_sb[:, ts], ps[:], RELU,
                        bias=b1_sb[:, 0:1], scale=scale_out,
                    )
                    # ff + residual + b2 inside one matmul group:
                    # psum = W2^T r + I h + b2 (x) ones
                    for c in range(C):
                        ps = psA.tile([P, FQ], F32, tag="main")
                        # residual first: it only needs h, so it runs while
                        # the relu output is still in flight
                        nc.tensor.matmul(
                            ps[:], ident_sb[:], h_sb[:, c, ts],
                            start=True, stop=False,
                        )
                        nc.tensor.matmul(
                            ps[:], w2_sb[:, c * P:(c + 1) * P], r_sb[:, ts],
                            start=False, stop=trivial_bias,
                        )
                        if not trivial_bias:
                            nc.tensor.matmul(
                                ps[:], b2r_sb[0:1, c * P:(c + 1) * P],
                                ones_row[0:1, ts], start=False, stop=True,
                            )
                        if c % 2 == 0:
                            nc.scalar.copy(t_sb[:, c, ts], ps[:])
                        else:
                            nc.vector.tensor_copy(t_sb[:, c, ts], ps[:])
                        nc.vector.tensor_mul(tsq_sb[:, c, ts], t_sb[:, c, ts],
                                             t_sb[:, c, ts])
                        if c == 1:
                            nc.vector.tensor_tensor(
                                stw_sb[:, 0, ts], t_sb[:, 0, ts],
                                t_sb[:, 1, ts], ADD)
                            nc.vector.tensor_tensor(
                                stw_sb[:, 1, ts], tsq_sb[:, 0, ts],
                                tsq_sb[:, 1, ts], ADD)
                        if c == 3:
                            nc.vector.tensor_tensor(
                                stw_sb[:, 0, ts], stw_sb[:, 0, ts],
                                t_sb[:, 2, ts], ADD)
                            nc.vector.tensor_tensor(
                                stw_sb[:, 0, ts], stw_sb[:, 0, ts],
                                t_sb[:, 3, ts], ADD)
                            nc.vector.tensor_tensor(
                                stw_sb[:, 1, ts], stw_sb[:, 1, ts],
                                tsq_sb[:, 2, ts], ADD)
                            nc.vector.tensor_tensor(
                                stw_sb[:, 1, ts], stw_sb[:, 1, ts],
                                tsq_sb[:, 3, ts], ADD)
                    # layer norm for this chunk: deferred (emitted after
                    # the next chunk's q-matmuls) so the PE never waits on
                    # the t copies
                    def _ln(nq=nq, last=last):
                        if trivial_ln:
                            fused_trivial_ln(t_sb, h_sb, nq, out_f32=last)
                        else:
                            layer_norm(t_sb, t_sb, g1_sb, be1_sb,
                                       mu1_bc, rstd1_bc, nq, use_stw=True)
                            layer_norm(t_sb, h_sb, g2_sb, be2_sb,
                                       mu2_bc, rstd2_bc, nq, out_f32=last)
                    pending_ln.append(_ln)
            while pending_ln:
                pending_ln.pop(0)()
    nc.compile()
    return nc


_NC_CACHE = {}


def _get_nc(trivial_ln, trivial_bias=False):
    key = ("nc", trivial_ln, trivial_bias)
    if key not in _NC_CACHE:
        _NC_CACHE[key] = build(trivial_ln=trivial_ln, trivial_bias=trivial_bias)
    return _NC_CACHE[key]


def _shard_inputs(x, Wq, bq, Wk, bk_, Wv, bv_, W1, b1, W2, b2, ln1_g, ln1_b, ln2_g, ln2_b):
    """Full inputs -> list of 8 per-core in_maps."""
    bf = ml_dtypes.bfloat16
    C = D // P
    SQ = S // 2
    shared = {
        "wq": np.ascontiguousarray(Wq.reshape(C, P, D)).astype(ml_dtypes.float8_e4m3),
        "wk": np.ascontiguousarray(Wk.reshape(C, P, D)).astype(ml_dtypes.float8_e4m3),
        "wv": np.ascontiguousarray(Wv.reshape(C, P, D)).astype(ml_dtypes.float8_e4m3),
        "w1": np.ascontiguousarray(W1.reshape(C, P, HID)).astype(bf),
        "w2": np.ascontiguousarray(W2).astype(bf),
        "bqc": np.ascontiguousarray(bq.reshape(C, P, 1)).astype(ml_dtypes.float8_e4m3),
        "bk": np.ascontiguousarray(bk_.reshape(C, P).T).astype(np.float32),
        "bv": np.ascontiguousarray(bv_.reshape(1, D)).astype(np.float32),
        "b1d": np.ascontiguousarray(b1.reshape(HID, 1)).astype(np.float32),
        "b2r": np.ascontiguousarray(b2.reshape(1, D)).astype(bf),
        "g1d": np.ascontiguousarray(ln1_g.reshape(C, P).T).astype(np.float32),
        "be1d": np.ascontiguousarray(ln1_b.reshape(C, P).T).astype(np.float32),
        "g2d": np.ascontiguousarray(ln2_g.reshape(C, P).T).astype(np.float32),
        "be2d": np.ascontiguousarray(ln2_b.reshape(C, P).T).astype(np.float32),
    }
    in_maps = []
    for core in range(8):
        b, j = core // 2, core % 2
        xT = np.ascontiguousarray(x[b].T)  # [D, S]
        m = dict(shared)
        m["xt"] = xT.reshape(C, P, S).astype(ml_dtypes.float8_e4m3)
        m["xq"] = np.ascontiguousarray(
            xT[:, j * SQ:(j + 1) * SQ].reshape(C, P, SQ)
        ).astype(bf)
        in_maps.append(m)
    return in_maps


def _gather_output(results):
    SQ = S // 2
    out = np.empty((B, S, D), np.float32)
    for core, res in enumerate(results):
        b, j = core // 2, core % 2
        # res["out"]: [C, P, SQ] = h^T chunks -> h slice [SQ, D]
        out[b, j * SQ:(j + 1) * SQ, :] = res["out"].reshape(D, SQ).T
    return out


def _ln_trivial(inputs):
    return bool(
        np.all(inputs["ln1_g"] == 1.0) and np.all(inputs["ln1_b"] == 0.0)
        and np.all(inputs["ln2_g"] == 1.0) and np.all(inputs["ln2_b"] == 0.0)
    )


def _bias_trivial(inputs):
    return bool(all(np.all(inputs[k] == 0.0) for k in ("bq", "b2")))


def kernel(**inputs):
    nc = _get_nc(trivial_ln=_ln_trivial(inputs), trivial_bias=_bias_trivial(inputs))
    in_maps = _shard_inputs(
        inputs["x"], inputs["Wq"], inputs["bq"], inputs["Wk"], inputs["bk"],
        inputs["Wv"], inputs["bv"], inputs["W1"], inputs["b1"], inputs["W2"],
        inputs["b2"], inputs["ln1_g"], inputs["ln1_b"], inputs["ln2_g"],
        inputs["ln2_b"],
    )
    res = run_bass_kernel_spmd(nc, in_maps, core_ids=list(range(8)))
    return _gather_output(res.results)

